# Initial kernel scaffold
#
"""Cox partial-likelihood loss on 8 Trainium2 NeuronCores.

reference:
    theta = hazard_pred.reshape(-1)                 # [n]
    R[i, j] = survtime[j] >= survtime[i]            # risk-set mask
    risk_sum[i] = sum_j exp(theta[j]) * R[i, j]
    loss = -mean((theta - log(risk_sum)) * censor)

Sharding: rows i are split across 8 cores (1024 rows each). Each core:
  - loads the full survtime/theta (32KB each),
  - computes exp(theta) once (ScalarE), casts to bf16,
  - builds the [8192 x 1024] row-chunk of the risk mask 128 j's at a
    time with DVE tensor_scalar compares (fp32 compare, bf16 0/1 out),
  - contracts mask chunks against exp(theta) on the TensorEngine,
    accumulating risk_sum for its 1024 rows in PSUM,
  - tail: Ln on ScalarE + fused (theta - ln)*censor reduction on DVE,
  - writes a single partial-sum scalar.
Host sums the 8 partials and applies -1/n.

j-index mapping: j = p*64 + c  (p = SBUF partition, c = chunk column),
so survtime/theta load as contiguous [128, 64] tiles and chunk c uses
column c as both the per-partition compare scalar and the matmul lhsT.
"""

from contextlib import ExitStack

import numpy as np

import concourse.bass as bass
import concourse.tile as tile
from concourse import mybir
from concourse.bass_utils import run_bass_kernel_spmd

DT = mybir.dt
N = 8192
CORES = 8
NL = N // CORES       # 1024 local rows per core
NCHUNK = 64           # j-chunks of 128
NHALF = NL // 2       # matmul free-dim limit is 512

_CACHE: dict = {}


def _build_nc() -> bass.Bass:
    nc = bass.Bass()
    st_all = nc.declare_dram_parameter("st_all", [N], DT.float32, isOutput=False)
    th_all = nc.declare_dram_parameter("th_all", [N], DT.float32, isOutput=False)
    st_loc = nc.declare_dram_parameter("st_loc", [NL], DT.float32, isOutput=False)
    th_loc = nc.declare_dram_parameter("th_loc", [NL], DT.float32, isOutput=False)
    cen_loc = nc.declare_dram_parameter("cen_loc", [NL], DT.float32, isOutput=False)
    partial = nc.declare_dram_parameter("partial", [1], DT.float32, isOutput=True)

    with ExitStack() as ctx, tile.TileContext(nc) as tc:
        const = ctx.enter_context(tc.tile_pool(name="const", bufs=1))
        masks = ctx.enter_context(tc.tile_pool(name="masks", bufs=4))
        psums = ctx.enter_context(tc.tile_pool(name="psums", bufs=1, space="PSUM"))
        tailp = ctx.enter_context(tc.tile_pool(name="tailp", bufs=1))

        # j-major tiles: [p, c] holds index j = p*64 + c
        st_sb = const.tile([128, NCHUNK], DT.float32)
        nc.sync.dma_start(out=st_sb, in_=st_all[:].rearrange("(p c) -> p c", c=NCHUNK))
        th_sb = const.tile([128, NCHUNK], DT.float32)
        nc.sync.dma_start(out=th_sb, in_=th_all[:].rearrange("(p c) -> p c", c=NCHUNK))

        e32 = const.tile([128, NCHUNK], DT.float32)
        nc.scalar.activation(out=e32, in_=th_sb, func=mybir.ActivationFunctionType.Exp)
        e16 = const.tile([128, NCHUNK], DT.bfloat16)
        nc.vector.tensor_copy(out=e16, in_=e32)

        # local survtime broadcast to all partitions (free dim = local row i)
        si_row = const.tile([1, NL], DT.float32)
        nc.sync.dma_start(out=si_row, in_=st_loc[:].rearrange("(o n) -> o n", o=1))
        si_b = const.tile([128, NL], DT.float32)
        nc.gpsimd.partition_broadcast(si_b, si_row)

        # tail inputs, overlapped with the main loop
        thl = tailp.tile([1, NL], DT.float32)
        nc.sync.dma_start(out=thl, in_=th_loc[:].rearrange("(o n) -> o n", o=1))
        cenl = tailp.tile([1, NL], DT.float32)
        nc.sync.dma_start(out=cenl, in_=cen_loc[:].rearrange("(o n) -> o n", o=1))
        thc = tailp.tile([1, NL], DT.float32)
        nc.vector.tensor_mul(thc, thl, cenl)
        thc_sum = tailp.tile([1, 1], DT.float32)
        nc.vector.tensor_reduce(
            out=thc_sum, in_=thc, axis=mybir.AxisListType.X, op=mybir.AluOpType.add
        )

        # main loop: risk_sum[i] = sum_j e[j] * (st[j] >= st_loc[i])
        p0 = psums.tile([1, NHALF], DT.float32, tag="p0")
        p1 = psums.tile([1, NHALF], DT.float32, tag="p1")
        for c in range(NCHUNK):
            m = masks.tile([128, NL], DT.bfloat16)
            nc.vector.tensor_scalar(
                out=m,
                in0=si_b,
                scalar1=st_sb[:, c : c + 1],
                scalar2=None,
                op0=mybir.AluOpType.is_le,
            )
            nc.tensor.matmul(
                p0, e16[:, c : c + 1], m[:, 0:NHALF],
                start=(c == 0), stop=(c == NCHUNK - 1),
            )
            nc.tensor.matmul(
                p1, e16[:, c : c + 1], m[:, NHALF:NL],
                start=(c == 0), stop=(c == NCHUNK - 1),
            )

        # tail: partial = sum_i (theta_i - ln(risk_i)) * censor_i
        lnt = tailp.tile([1, NL], DT.float32)
        nc.scalar.activation(
            out=lnt[:, 0:NHALF], in_=p0, func=mybir.ActivationFunctionType.Ln
        )
        nc.scalar.activation(
            out=lnt[:, NHALF:NL], in_=p1, func=mybir.ActivationFunctionType.Ln
        )
        junk = tailp.tile([1, NL], DT.float32)
        lc_sum = tailp.tile([1, 1], DT.float32)
        nc.vector.tensor_tensor_reduce(
            out=junk,
            in0=lnt,
            in1=cenl,
            scale=-1.0,
            scalar=0.0,
            op0=mybir.AluOpType.mult,
            op1=mybir.AluOpType.add,
            accum_out=lc_sum,
        )
        res = tailp.tile([1, 1], DT.float32)
        nc.vector.tensor_add(res, thc_sum, lc_sum)
        nc.sync.dma_start(out=partial[:].rearrange("(o n) -> o n", o=1), in_=res)

    return nc


def _get_nc() -> bass.Bass:
    if "nc" not in _CACHE:
        _CACHE["nc"] = _build_nc()
    return _CACHE["nc"]


def make_in_maps(survtime: np.ndarray, theta: np.ndarray, censor: np.ndarray):
    st = np.ascontiguousarray(survtime, dtype=np.float32)
    th = np.ascontiguousarray(theta, dtype=np.float32).reshape(-1)
    cen = np.ascontiguousarray(censor, dtype=np.float32)
    in_maps = []
    for k in range(CORES):
        lo, hi = k * NL, (k + 1) * NL
        in_maps.append(
            {
                "st_all": st,
                "th_all": th,
                "st_loc": st[lo:hi].copy(),
                "th_loc": th[lo:hi].copy(),
                "cen_loc": cen[lo:hi].copy(),
            }
        )
    return in_maps


def kernel(hazard_pred: np.ndarray, survtime: np.ndarray, censor: np.ndarray):
    nc = _get_nc()
    in_maps = make_in_maps(survtime, hazard_pred, censor)
    out = run_bass_kernel_spmd(nc, in_maps, list(range(CORES)))
    partials = np.array(
        [np.asarray(out.results[k]["partial"]).reshape(-1)[0] for k in range(CORES)],
        dtype=np.float64,
    )
    return np.float32(-partials.sum() / N)


# revision 9
# speedup vs baseline: 4.6286x; 4.6286x over previous
"""Cox partial-likelihood loss on 8 Trainium2 NeuronCores.

reference:
    theta = hazard_pred.reshape(-1)                 # [n]
    R[i, j] = survtime[j] >= survtime[i]            # risk-set mask
    risk_sum[i] = sum_j exp(theta[j]) * R[i, j]
    loss = -mean((theta - log(risk_sum)) * censor)

Sharding: rows i are split across 8 cores (1024 rows each). Each core:
  - loads the full survtime/theta (32KB each),
  - computes exp(theta) once (ScalarE), casts to bf16,
  - builds the [8192 x 1024] row-chunk of the risk mask 128 j's at a
    time with DVE tensor_scalar compares (fp32 compare, bf16 0/1 out),
  - contracts mask chunks against exp(theta) on the TensorEngine,
    accumulating risk_sum for its 1024 rows in PSUM,
  - tail: Ln on ScalarE + (theta - ln)*censor reduction on DVE,
  - writes a single partial-sum scalar.
Host sums the 8 partials and applies -1/n.

j-index mapping: j = p*64 + c  (p = SBUF partition, c = chunk column),
so survtime/theta load as contiguous [128, 64] tiles and chunk c uses
column c as both the per-partition compare scalar and the matmul lhsT.
"""

from contextlib import ExitStack, nullcontext

import numpy as np

import concourse.bacc as bacc
import concourse.bass as bass
import concourse.tile as tile
from concourse import mybir
from concourse.bass_utils import run_bass_kernel_spmd

DT = mybir.dt
N = 8192
CORES = 8
NL = N // CORES       # 1024 local rows per core
NCHUNK = 64           # j-chunks of 128
NHALF = NL // 2       # matmul free-dim limit is 512

_CACHE: dict = {}


def _emit_body(nc, const, masks, psums, tailp, st_all, th_all, st_loc, th_loc,
               cen_loc, partial):
    # j-major tiles: [p, c] holds index j = p*64 + c
    st_sb = const.tile([128, NCHUNK], DT.float32)
    nc.sync.dma_start(out=st_sb, in_=st_all[:].rearrange("(p c) -> p c", c=NCHUNK))
    th_sb = const.tile([128, NCHUNK], DT.float32)
    nc.sync.dma_start(out=th_sb, in_=th_all[:].rearrange("(p c) -> p c", c=NCHUNK))

    e32 = const.tile([128, NCHUNK], DT.float32)
    nc.scalar.activation(out=e32, in_=th_sb, func=mybir.ActivationFunctionType.Exp)
    e16 = const.tile([128, NCHUNK], DT.bfloat16)
    nc.vector.tensor_copy(out=e16, in_=e32)

    # local survtime broadcast to all partitions (free dim = local row i)
    si_b = const.tile([128, NL], DT.float32)
    st_loc_row = st_loc[:].rearrange("(o n) -> o n", o=1)
    nc.gpsimd.dma_start(out=si_b, in_=st_loc_row.partition_broadcast(128))

    # tail inputs, overlapped with the main loop
    thl = tailp.tile([1, NL], DT.float32)
    nc.sync.dma_start(out=thl, in_=th_loc[:].rearrange("(o n) -> o n", o=1))
    cenl = tailp.tile([1, NL], DT.float32)
    nc.sync.dma_start(out=cenl, in_=cen_loc[:].rearrange("(o n) -> o n", o=1))
    thc = tailp.tile([1, NL], DT.float32)
    nc.vector.tensor_mul(thc, thl, cenl)
    thc_sum = tailp.tile([1, 1], DT.float32)
    nc.vector.tensor_reduce(
        out=thc_sum, in_=thc, axis=mybir.AxisListType.X, op=mybir.AluOpType.add
    )

    # main loop: risk_sum[i] = sum_j e[j] * (st[j] >= st_loc[i])
    p0 = psums.tile([1, NHALF], DT.float32, tag="p0")
    p1 = psums.tile([1, NHALF], DT.float32, tag="p1")
    for c in range(NCHUNK):
        m = masks.tile([128, NL], DT.bfloat16)
        nc.vector.tensor_scalar(
            out=m,
            in0=si_b,
            scalar1=st_sb[:, c : c + 1],
            scalar2=None,
            op0=mybir.AluOpType.is_le,
        )
        nc.tensor.matmul(
            p0, e16[:, c : c + 1], m[:, 0:NHALF],
            start=(c == 0), stop=(c == NCHUNK - 1),
        )
        nc.tensor.matmul(
            p1, e16[:, c : c + 1], m[:, NHALF:NL],
            start=(c == 0), stop=(c == NCHUNK - 1),
        )

    # tail: partial = sum_i (theta_i - ln(risk_i)) * censor_i
    lnt = tailp.tile([1, NL], DT.float32)
    nc.scalar.activation(
        out=lnt[:, 0:NHALF], in_=p0, func=mybir.ActivationFunctionType.Ln
    )
    nc.scalar.activation(
        out=lnt[:, NHALF:NL], in_=p1, func=mybir.ActivationFunctionType.Ln
    )
    lnc = tailp.tile([1, NL], DT.float32)
    nc.vector.tensor_mul(lnc, lnt, cenl)
    lc_sum = tailp.tile([1, 1], DT.float32)
    nc.vector.tensor_reduce(
        out=lc_sum, in_=lnc, axis=mybir.AxisListType.X, op=mybir.AluOpType.add
    )
    res = tailp.tile([1, 1], DT.float32)
    nc.vector.tensor_sub(res, thc_sum, lc_sum)
    nc.sync.dma_start(out=partial[:].rearrange("(o n) -> o n", o=1), in_=res)


def _build_nc(reps: int | None = None) -> bass.Bass:
    nc = bacc.Bacc()
    st_all = nc.declare_dram_parameter("st_all", [N], DT.float32, isOutput=False)
    th_all = nc.declare_dram_parameter("th_all", [N], DT.float32, isOutput=False)
    st_loc = nc.declare_dram_parameter("st_loc", [NL], DT.float32, isOutput=False)
    th_loc = nc.declare_dram_parameter("th_loc", [NL], DT.float32, isOutput=False)
    cen_loc = nc.declare_dram_parameter("cen_loc", [NL], DT.float32, isOutput=False)
    partial = nc.declare_dram_parameter("partial", [1], DT.float32, isOutput=True)

    with tile.TileContext(nc) as tc, ExitStack() as ctx:
        const = ctx.enter_context(tc.tile_pool(name="const", bufs=1))
        masks = ctx.enter_context(tc.tile_pool(name="masks", bufs=4))
        psums = ctx.enter_context(tc.tile_pool(name="psums", bufs=1, space="PSUM"))
        tailp = ctx.enter_context(tc.tile_pool(name="tailp", bufs=1))

        loop = tc.For_i(0, reps, 1) if reps is not None else nullcontext()
        with loop:
            _emit_body(nc, const, masks, psums, tailp, st_all, th_all, st_loc,
                       th_loc, cen_loc, partial)

    nc.compile()
    return nc


def _get_nc() -> bass.Bass:
    if "nc" not in _CACHE:
        _CACHE["nc"] = _build_nc()
    return _CACHE["nc"]


def make_in_maps(survtime: np.ndarray, theta: np.ndarray, censor: np.ndarray):
    st = np.ascontiguousarray(survtime, dtype=np.float32)
    th = np.ascontiguousarray(theta, dtype=np.float32).reshape(-1)
    cen = np.ascontiguousarray(censor, dtype=np.float32)
    in_maps = []
    for k in range(CORES):
        lo, hi = k * NL, (k + 1) * NL
        in_maps.append(
            {
                "st_all": st,
                "th_all": th,
                "st_loc": st[lo:hi].copy(),
                "th_loc": th[lo:hi].copy(),
                "cen_loc": cen[lo:hi].copy(),
            }
        )
    return in_maps


def kernel(hazard_pred: np.ndarray, survtime: np.ndarray, censor: np.ndarray):
    nc = _get_nc()
    in_maps = make_in_maps(survtime, hazard_pred, censor)
    out = run_bass_kernel_spmd(nc, in_maps, list(range(CORES)))
    partials = np.array(
        [np.asarray(out.results[k]["partial"]).reshape(-1)[0] for k in range(CORES)],
        dtype=np.float64,
    )
    return np.float32(-partials.sum() / N)
